# revision 25
# baseline (speedup 1.0000x reference)
"""Distributed GCN(4-layer) + LSTM readout kernel for 8 TRN2 NeuronCores.

Self-contained: hardcodes the problem shapes (N=50000, E=800000, D=H=128,
G=500 graphs x L=100 nodes, C=10) and the 8-way sharding.

Strategy (v2)
-------------
- Nodes sharded contiguously across 8 cores at graph boundaries
  (6300 x4 + 6200 x4); per-graph LSTM readout is purely local.
- Per GCN layer each core computes u = t @ W for its shard, writes the slab
  to DRAM and AllGathers the 8 slabs into a replicated bf16 table.
- Edge aggregation as PSUM matmul accumulation over 128-edge chunks:
  psum[f,d] += sum_e GX[e,f] * S[e,d].  GX rows come from a dma_gather in
  prepare_only mode (descriptor gen on GpSimd, transfer fired by
  trigger_dma so the engine never blocks on the DMA).  S is a
  host-precomputed "staircase" (one-hot rows scaled by a[dst]^2 for layers
  0-2 / a[dst] for layer 3), which folds the dst-side GCN normalization and
  the next layer's a-prescale into the matmul: the psum is relu'd straight
  into the next layer's input.  Self-loops enter as a diag(a^2 | a) matmul
  of the local slab block.
- dma_gather indices are int16, so the table is addressed in two halves
  (cores 0-3 / 4-7) and per-block edge lists are split accordingly.
- LSTM: x-projections for all timesteps are big matmuls done right after
  layer 4 (gate bias folded in during the psum->SBUF copy).  The recurrence
  uses bf16 W_hh, gates reordered [i,f,o,g], and sigmoid-only activations
  (tanh(x) = 2*sigmoid(2x)-1) to avoid activation-table reloads.
"""
import dataclasses
import numpy as np
import ml_dtypes

import concourse.bass as bass
import concourse.mybir as mybir
import concourse.tile as tile
from concourse import bacc
from concourse.bass_utils import run_bass_kernel_spmd

F32 = mybir.dt.float32
BF16 = mybir.dt.bfloat16
I16 = mybir.dt.int16
P = 128

TRACE = False          # set True (e.g. from test.py) to profile
LAST_RESULTS = None    # BassKernelResults of the last run (for profiling)
DEBUG_DUMPS = False    # add t1/z4 debug outputs


@dataclasses.dataclass
class Config:
    N: int = 50000
    E: int = 800000
    D: int = 128
    H: int = 128
    L: int = 100
    C: int = 10
    NCORES: int = 8
    GROUP_BLOCKS: int = 3  # dst blocks per gather super-group
    PIPE: int = 2          # groups of gather-prep lookahead

    def __post_init__(self):
        assert self.D == 128 and self.H == 128
        base = (self.N // self.NCORES) // self.L * self.L
        hi = base + self.L
        n_hi = (self.N - base * self.NCORES) // self.L
        self.sizes = [hi] * n_hi + [base] * (self.NCORES - n_hi)
        assert sum(self.sizes) == self.N
        self.offs = np.concatenate([[0], np.cumsum(self.sizes)]).astype(np.int64)
        self.S_PAD = hi
        self.NBLK = -(-self.S_PAD // P)
        self.SLAB = self.NBLK * P
        self.THALF = (self.NCORES // 2) * self.SLAB
        assert self.THALF <= 32768, "int16 gather index overflow"
        self.NG = self.S_PAD // self.L
        self.G = self.N // self.L


def preprocess(cfg, x, edge_index, Ws, bs, W_ih, W_hh, b_ih, b_hh,
               lin_W, lin_b):
    N = cfg.N
    src = np.asarray(edge_index[0], np.int64)
    dst = np.asarray(edge_index[1], np.int64)
    deg = (np.bincount(dst, minlength=N) + 1.0).astype(np.float32)
    a = (1.0 / np.sqrt(deg)).astype(np.float32)

    shard_of = np.searchsorted(cfg.offs[1:], np.arange(N), side="right")
    trow = shard_of * cfg.SLAB + (np.arange(N) - cfg.offs[shard_of])

    e_core = shard_of[dst]
    e_half = (trow[src] >= cfg.THALF).astype(np.int64)
    e_tix = (trow[src] - e_half * cfg.THALF).astype(np.int64)
    e_blk = ((dst - cfg.offs[e_core]) // P).astype(np.int64)
    e_seg = ((dst - cfg.offs[e_core]) % P).astype(np.int64)

    order = np.lexsort((dst, e_blk, e_half, e_core))
    src_s = src[order]
    dst_s = dst[order]
    core_s, half_s, tix_s, blk_s, seg_s = (
        arr[order] for arr in (e_core, e_half, e_tix, e_blk, e_seg))

    counts = np.zeros((cfg.NCORES, 2, cfg.NBLK), np.int64)
    np.add.at(counts, (core_s, half_s, blk_s), 1)
    chunks = -(-counts.max(axis=0) // P)
    cA, cB = chunks[0], chunks[1]

    groups = []
    for g0 in range(0, cfg.NBLK, cfg.GROUP_BLOCKS):
        groups.append(list(range(g0, min(g0 + cfg.GROUP_BLOCKS, cfg.NBLK))))

    TA = int(cA.sum()) * P
    TB = int(cB.sum()) * P
    run_off = np.zeros((cfg.NCORES, 2, cfg.NBLK), np.int64)
    run_off.reshape(-1)[1:] = np.cumsum(counts.reshape(-1))[:-1]

    # LSTM gate reorder [i, f, g, o] -> [i, f, o, g] so sigmoids are
    # contiguous; fold b_ih + b_hh into the x-projection copy.  The g-gate
    # rows are pre-scaled by 2 so tanh(x) = 2*sigmoid(2x)-1 needs no
    # separate scaled activation: one sigmoid covers all four gates.
    perm = [0, 1, 3, 2]
    gscale = [1.0, 1.0, 1.0, 2.0]
    H = cfg.H
    WihR = np.concatenate(
        [sc * np.asarray(W_ih, np.float32)[q * H:(q + 1) * H]
         for q, sc in zip(perm, gscale)], axis=0)
    WhhR = np.concatenate(
        [sc * np.asarray(W_hh, np.float32)[q * H:(q + 1) * H]
         for q, sc in zip(perm, gscale)], axis=0)
    bgR = np.stack([sc * (np.asarray(b_ih, np.float32)
                          + np.asarray(b_hh, np.float32))[q * H:(q + 1) * H]
                    for q, sc in zip(perm, gscale)], axis=0)  # [4, H]

    in_maps = []
    for c in range(cfg.NCORES):
        idx_flat = {0: np.zeros(TA, np.int64), 1: np.zeros(TB, np.int64)}
        seg_flat = {0: np.full(TA, -1, np.int64),
                    1: np.full(TB, -1, np.int64)}
        adst_flat = {0: np.zeros(TA, np.float32), 1: np.zeros(TB, np.float32)}
        for h_i, tot_c in enumerate((cA, cB)):
            pos = 0
            for b in range(cfg.NBLK):
                n = int(counts[c, h_i, b])
                o = int(run_off[c, h_i, b])
                idx_flat[h_i][pos:pos + n] = tix_s[o:o + n]
                seg_flat[h_i][pos:pos + n] = seg_s[o:o + n]
                adst_flat[h_i][pos:pos + n] = a[dst_s[o:o + n]]
                pos += int(tot_c[b]) * P

        idxA = np.zeros((P, max(TA // 16, 1)), np.int16)
        idxB = np.zeros((P, max(TB // 16, 1)), np.int16)
        if TA:
            idxA[:] = np.tile(idx_flat[0].reshape(-1, 16).T.astype(np.int16),
                              (8, 1))
        if TB:
            idxB[:] = np.tile(idx_flat[1].reshape(-1, 16).T.astype(np.int16),
                              (8, 1))

        # seg columns for the on-chip staircase build (pad -> 255 never
        # matches iota, giving an all-zero staircase row)
        segA = np.zeros((P, max(TA // P, 1)), ml_dtypes.bfloat16)
        segB = np.zeros((P, max(TB // P, 1)), ml_dtypes.bfloat16)
        if TA:
            sf = np.where(seg_flat[0] >= 0, seg_flat[0], 255).astype(
                np.float32)
            segA[:] = sf.reshape(-1, P).T.astype(ml_dtypes.bfloat16)
        if TB:
            sf = np.where(seg_flat[1] >= 0, seg_flat[1], 255).astype(
                np.float32)
            segB[:] = sf.reshape(-1, P).T.astype(ml_dtypes.bfloat16)

        o, s = int(cfg.offs[c]), int(cfg.sizes[c])
        xT = np.zeros((P, cfg.SLAB), np.float32)
        xT[:, :s] = np.asarray(x[o:o + s], np.float32).T
        a_b = np.ones((P, cfg.SLAB), ml_dtypes.bfloat16)
        a_b[:, :s] = np.tile(a[o:o + s], (P, 1)).astype(ml_dtypes.bfloat16)
        a2_b = np.ones((P, cfg.SLAB), ml_dtypes.bfloat16)
        a2_b[:, :s] = np.tile(a[o:o + s] ** 2, (P, 1)).astype(
            ml_dtypes.bfloat16)

        m = {
            "xT": xT, "a_b": a_b, "a2_b": a2_b,
            "idxA": idxA, "idxB": idxB, "segA": segA, "segB": segB,
            "iota": np.tile(np.arange(P, dtype=np.float32), (P, 1)).astype(
                ml_dtypes.bfloat16),
            "ident": np.eye(P, dtype=ml_dtypes.bfloat16),
            "WihT": np.ascontiguousarray(WihR.T).astype(ml_dtypes.bfloat16),
            "WhhT": np.ascontiguousarray(WhhR.T).astype(ml_dtypes.bfloat16),
            "bg": np.ascontiguousarray(bgR.T),  # [H, 4] f32
            "linW": np.asarray(lin_W, np.float32).astype(ml_dtypes.bfloat16),
            "linb": np.tile(np.asarray(lin_b, np.float32), (P, 1)),
        }
        for li in range(4):
            m[f"W{li}"] = np.asarray(Ws[li], np.float32).astype(
                ml_dtypes.bfloat16)
        in_maps.append(m)

    sched = dict(cA=cA.astype(np.int64), cB=cB.astype(np.int64),
                 groups=groups, TA=TA, TB=TB,
                 bias_zero=all(not np.any(np.asarray(b)) for b in bs))
    return in_maps, sched


def build_program(cfg, sched, trn_type="TRN2", debug=False):
    nc = bacc.Bacc(trn_type, target_bir_lowering=False, debug=debug,
                   num_devices=cfg.NCORES, num_swdge_queues=4)
    cA, cB, groups = sched["cA"], sched["cB"], sched["groups"]
    TA, TB = sched["TA"], sched["TB"]
    assert sched["bias_zero"], "nonzero GCN bias not supported in v2"
    SLAB, NBLK, THALF, NG = cfg.SLAB, cfg.NBLK, cfg.THALF, cfg.NG
    NGRP = len(groups)

    def din(name, shape, dt):
        return nc.dram_tensor(name, shape, dt, kind="ExternalInput")

    xT_d = din("xT", [P, SLAB], F32)
    a_d = din("a_b", [P, SLAB], BF16)
    a2_d = din("a2_b", [P, SLAB], BF16)
    idxA_d = din("idxA", [P, max(TA // 16, 1)], I16)
    idxB_d = din("idxB", [P, max(TB // 16, 1)], I16)
    segA_d = din("segA", [P, max(TA // P, 1)], BF16)
    segB_d = din("segB", [P, max(TB // P, 1)], BF16)
    iota_d = din("iota", [P, P], BF16)
    ident_d = din("ident", [P, P], BF16)
    W_d = [din(f"W{li}", [P, P], BF16) for li in range(4)]
    WihT_d = din("WihT", [P, 4 * P], BF16)
    WhhT_d = din("WhhT", [P, 4 * P], BF16)
    bg_d = din("bg", [P, 4], F32)
    linW_d = din("linW", [P, cfg.C], BF16)
    linb_d = din("linb", [P, cfg.C], F32)
    out_d = nc.dram_tensor("out", [NG, cfg.C], F32, kind="ExternalOutput")
    if DEBUG_DUMPS:
        t1_d = nc.dram_tensor("t1_dbg", [P, SLAB], BF16,
                              kind="ExternalOutput")
        z4_d = nc.dram_tensor("z4_dbg", [P, SLAB], BF16,
                              kind="ExternalOutput")
        gx_d = nc.dram_tensor("gx_dbg", [P, 4 * SLAB], BF16,
                              kind="ExternalOutput")

    rg = [list(range(cfg.NCORES))]
    qsem = [nc.alloc_semaphore(f"gq{q}") for q in range(4)]

    with tile.TileContext(nc) as tc:
        with tc.tile_pool(name="dram", bufs=1, space="DRAM") as dpool, \
             tc.tile_pool(name="const", bufs=1) as cpool, \
             tc.tile_pool(name="state", bufs=1) as spool, \
             tc.tile_pool(name="work", bufs=2) as wpool, \
             tc.tile_pool(name="gwork", bufs=2) as gpool, \
             tc.tile_pool(name="psum", bufs=4, space="PSUM") as ppool:

            def cload(dram, shape, dt, tag):
                t = cpool.tile(shape, dt, tag=tag)
                nc.sync.dma_start(t[:], dram[:])
                return t

            iota_t = cload(iota_d, [P, P], BF16, "c_iota")
            ident_t = cload(ident_d, [P, P], BF16, "c_ident")
            W_t = [cload(W_d[i], [P, P], BF16, f"c_W{i}") for i in range(4)]
            WihT_t = cload(WihT_d, [P, 4 * P], BF16, "c_wih")
            WhhT_t = cload(WhhT_d, [P, 4 * P], BF16, "c_whh")
            bg_t = cload(bg_d, [P, 4], F32, "c_bg")
            linW_t = cload(linW_d, [P, cfg.C], BF16, "c_linw")
            linb_t = cload(linb_d, [P, cfg.C], F32, "c_linb")

            a_t = spool.tile([P, SLAB], BF16, tag="a")
            nc.sync.dma_start(a_t[:], a_d[:])
            a2_t = spool.tile([P, SLAB], BF16, tag="a2")
            nc.sync.dma_start(a2_t[:], a2_d[:])

            t_even = spool.tile([P, SLAB], BF16, tag="t_even")
            t_big = spool.tile([P, SLAB], BF16, tag="t_big")
            gx_nodes = spool.tile([P, 4 * SLAB], BF16, tag="gx_nodes")
            slab_t = spool.tile([P, SLAB], BF16, tag="slab")

            for j in range(0, SLAB, 512):
                w = min(512, SLAB - j)
                xc = wpool.tile([P, 512], F32, tag="xchunk")
                nc.sync.dma_start(xc[:, :w], xT_d[:, j:j + w])
                nc.vector.tensor_tensor(
                    out=t_even[:, j:j + w], in0=xc[:, :w], in1=a_t[:, j:j + w],
                    op=mybir.AluOpType.mult)

            for li in range(4):
                cur = t_even if li % 2 == 0 else t_big
                nxt = t_big if li % 2 == 0 else t_even
                an_t = a2_t if li < 3 else a_t

                slab_dram = dpool.tile([SLAB, P], BF16, tag="slab_dram",
                                       bufs=2)
                table_dram = dpool.tile([cfg.NCORES * SLAB, P], BF16,
                                        addr_space="Shared", tag="table",
                                        bufs=2)

                # ---- u = t @ W -> slab [node, feat] ----
                for k in range(NBLK):
                    pu = ppool.tile([P, P], F32, tag="mm", space="PSUM")
                    nc.tensor.matmul(pu[:], lhsT=cur[:, k * P:(k + 1) * P],
                                     rhs=W_t[li][:], start=True, stop=True)
                    nc.vector.tensor_copy(out=slab_t[:, k * P:(k + 1) * P],
                                          in_=pu[:])
                nc.sync.dma_start(
                    slab_dram[:].rearrange("(b p) f -> p b f", p=P),
                    slab_t[:].rearrange("p (b f) -> p b f", f=P))
                nc.gpsimd.collective_compute(
                    "AllGather", mybir.AluOpType.bypass,
                    replica_groups=rg,
                    ins=[slab_dram[:]],
                    outs=[table_dram[:]],
                )

                # ---- edge aggregation: software-pipelined groups ----
                # stage A (prep): load idx/stair/diag tiles, generate gather
                # descriptors (prepare_only).  stage B (fire+consume): trigger
                # the DMAs (waits on the AllGather via deferred deps), then
                # matmul-accumulate and relu into `nxt`.
                st_off = [0, 0]     # column offsets into stair/idx tensors
                st_off_c = [0, 0]
                pend = []           # (blks, gx{}, st{}, dg, counts)
                ao_t = 0

                def stage_prep(gi):
                    blks = groups[gi]
                    q = gi % 4
                    cnts = {0: int(cA[blks].sum()), 1: int(cB[blks].sum())}
                    gx, st = {}, {}
                    for half, idxd, segd in ((0, idxA_d, segA_d),
                                             (1, idxB_d, segB_d)):
                        ncnt = cnts[half]
                        if ncnt == 0:
                            continue
                        off = st_off[half]
                        it = gpool.tile([P, ncnt * 8], I16, tag=f"idx{half}")
                        nc.sync.dma_start(
                            it[:], idxd[:, off * 8:(off + ncnt) * 8])
                        sg_c = gpool.tile([P, ncnt], BF16, tag=f"seg{half}")
                        nc.sync.dma_start(sg_c[:],
                                          segd[:, off:off + ncnt])
                        s_t = gpool.tile([P, ncnt, P], BF16, tag=f"st{half}")
                        nc.vector.tensor_tensor(
                            out=s_t[:],
                            in0=sg_c[:].rearrange("p (c o) -> p c o", o=1)
                                .to_broadcast((P, ncnt, P)),
                            in1=iota_t[:].rearrange("p (o f) -> p o f", o=1)
                                .to_broadcast((P, ncnt, P)),
                            op=mybir.AluOpType.is_equal)
                        g = gpool.tile([P, ncnt, P], BF16, tag=f"gx{half}")
                        nc.gpsimd.dma_gather(
                            out_ap=g[:],
                            in_ap=table_dram[half * THALF:(half + 1) * THALF, :],
                            idxs_ap=it[:],
                            num_idxs=ncnt * P,
                            num_idxs_reg=ncnt * P,
                            elem_size=P,
                            single_packet=False,
                            queue_num=q,
                        )
                        st_off[half] += ncnt
                        gx[half] = g
                        st[half] = s_t
                    pend.append((blks, gx, st, cnts))

                def stage_fire(gi):
                    blks, gx, st, cnts = pend.pop(0)
                    ca_in, cb_in = 0, 0
                    for bj, b in enumerate(blks):
                        pb = ppool.tile([P, P], F32, tag="mm", space="PSUM")
                        na, nb_ = int(cA[b]), int(cB[b])
                        nc.tensor.matmul(pb[:],
                                         lhsT=slab_t[:, b * P:(b + 1) * P],
                                         rhs=ident_t[:], start=True,
                                         stop=(na + nb_ == 0))
                        done = 0
                        for half, cnt, base in ((0, na, ca_in),
                                                (1, nb_, cb_in)):
                            for ci in range(cnt):
                                col = base + ci
                                done += 1
                                nc.tensor.matmul(
                                    pb[:], lhsT=gx[half][:, col, :],
                                    rhs=st[half][:, col, :],
                                    start=False, stop=(done == na + nb_))
                        ca_in += na
                        cb_in += nb_
                        nc.vector.scalar_tensor_tensor(
                            out=nxt[:, b * P:(b + 1) * P], in0=pb[:],
                            scalar=0.0, in1=an_t[:, b * P:(b + 1) * P],
                            op0=mybir.AluOpType.max,
                            op1=mybir.AluOpType.mult)

                for gi in range(NGRP + cfg.PIPE):
                    if gi < NGRP:
                        stage_prep(gi)
                    if gi >= cfg.PIPE:
                        stage_fire(gi - cfg.PIPE)

                if DEBUG_DUMPS and li == 0:
                    nc.sync.dma_start(t1_d[:], nxt[:])

            # ---- LSTM x-projections (z4 = t_even), bias folded in ----
            z4 = t_even
            if DEBUG_DUMPS:
                nc.sync.dma_start(z4_d[:], z4[:])
            for q in range(4):
                for j in range(0, SLAB, 512):
                    w = min(512, SLAB - j)
                    pgx = ppool.tile([P, 512], F32, tag="proj", space="PSUM",
                                     bufs=2)
                    nc.tensor.matmul(
                        pgx[:, :w], lhsT=WihT_t[:, q * P:(q + 1) * P],
                        rhs=z4[:, j:j + w], start=True, stop=True)
                    nc.scalar.activation(
                        out=gx_nodes[:, q * SLAB + j:q * SLAB + j + w],
                        in_=pgx[:, :w],
                        func=mybir.ActivationFunctionType.Identity,
                        bias=bg_t[:, q:q + 1])

            if DEBUG_DUMPS:
                nc.sync.dma_start(gx_d[:], gx_nodes[:])

            # ---- LSTM recurrence: sigmoid-only activations ----
            c_t = spool.tile([P, NG], F32, tag="c")
            h_t = spool.tile([P, NG], BF16, tag="h")
            nc.vector.memset(c_t[:], 0.0)
            nc.vector.memset(h_t[:], 0.0)

            gx3 = gx_nodes[:].rearrange("p (q n) -> p q n", q=4)
            SIG = mybir.ActivationFunctionType.Sigmoid
            for t in range(cfg.L):
                pg = ppool.tile([P, 4 * NG], F32, tag="lstm", space="PSUM",
                                bufs=2)
                for q in range(4):
                    nc.tensor.matmul(
                        pg[:, q * NG:(q + 1) * NG],
                        lhsT=WhhT_t[:, q * P:(q + 1) * P],
                        rhs=h_t[:], start=True, stop=True)
                gsum = wpool.tile([P, 4 * NG], F32, tag="gsum")
                nc.vector.tensor_tensor(
                    out=gsum[:].rearrange("p (q n) -> p q n", q=4),
                    in0=pg[:].rearrange("p (q n) -> p q n", q=4),
                    in1=gx3[:, :, t:cfg.S_PAD:cfg.L],
                    op=mybir.AluOpType.add)
                # gates [i | f | o | g]; g-rows host-scaled by 2 so one
                # sigmoid covers everything (tanh(x) = 2*sigmoid(2x)-1)
                s_all = wpool.tile([P, 4 * NG], F32, tag="sall")
                nc.scalar.activation(out=s_all[:], in_=gsum[:], func=SIG)
                i_t = s_all[:, :NG]
                f_t = s_all[:, NG:2 * NG]
                o_t = s_all[:, 2 * NG:3 * NG]
                s_g = s_all[:, 3 * NG:]
                t1 = wpool.tile([P, NG], F32, tag="t1")
                nc.vector.tensor_tensor(out=t1[:], in0=i_t, in1=s_g,
                                        op=mybir.AluOpType.mult)
                ig = wpool.tile([P, NG], F32, tag="ig")
                nc.vector.scalar_tensor_tensor(
                    out=ig[:], in0=t1[:], scalar=2.0, in1=i_t,
                    op0=mybir.AluOpType.mult, op1=mybir.AluOpType.subtract)
                fc = wpool.tile([P, NG], F32, tag="fc")
                nc.vector.tensor_tensor(out=fc[:], in0=f_t, in1=c_t[:],
                                        op=mybir.AluOpType.mult)
                nc.vector.tensor_tensor(out=c_t[:], in0=fc[:], in1=ig[:],
                                        op=mybir.AluOpType.add)
                s_c = wpool.tile([P, NG], F32, tag="sc")
                nc.scalar.activation(out=s_c[:], in_=c_t[:], func=SIG,
                                     scale=2.0)
                t2 = wpool.tile([P, NG], F32, tag="t2")
                nc.vector.tensor_tensor(out=t2[:], in0=o_t, in1=s_c[:],
                                        op=mybir.AluOpType.mult)
                nc.vector.scalar_tensor_tensor(
                    out=h_t[:], in0=t2[:], scalar=2.0, in1=o_t,
                    op0=mybir.AluOpType.mult, op1=mybir.AluOpType.subtract)

            po = ppool.tile([P, cfg.C], F32, tag="lstm", space="PSUM",
                            bufs=2)
            nc.tensor.matmul(po[:NG, :], lhsT=h_t[:, :NG], rhs=linW_t[:],
                             start=True, stop=True)
            os_ = wpool.tile([P, cfg.C], F32, tag="outs")
            nc.vector.tensor_tensor(out=os_[:NG, :], in0=po[:NG, :],
                                    in1=linb_t[:NG, :],
                                    op=mybir.AluOpType.add)
            nc.sync.dma_start(out_d[:], os_[:NG, :])

    nc.compile()
    return nc


def assemble(cfg, results):
    out = np.zeros((cfg.G, cfg.C), np.float32)
    for c in range(cfg.NCORES):
        g0 = int(cfg.offs[c]) // cfg.L
        ng = cfg.sizes[c] // cfg.L
        out[g0:g0 + ng] = results[c]["out"][:ng]
    return out


_BUILD_CACHE = {}


def kernel(x, edge_index, batch, W1, b1, W2, b2, W3, b3, W4, b4,
           W_ih, W_hh, b_ih, b_hh, lin_W, lin_b):
    global LAST_RESULTS
    cfg = Config()
    x = np.asarray(x, np.float32)
    edge_index = np.asarray(edge_index, np.int64)
    Ws = [np.asarray(w, np.float32) for w in (W1, W2, W3, W4)]
    bs = [np.asarray(b, np.float32) for b in (b1, b2, b3, b4)]

    in_maps, sched = preprocess(
        cfg, x, edge_index, Ws, bs,
        np.asarray(W_ih, np.float32), np.asarray(W_hh, np.float32),
        np.asarray(b_ih, np.float32), np.asarray(b_hh, np.float32),
        np.asarray(lin_W, np.float32), np.asarray(lin_b, np.float32))

    key = (sched["TA"], sched["TB"], tuple(sched["cA"]), tuple(sched["cB"]),
           sched["bias_zero"], DEBUG_DUMPS)
    if key not in _BUILD_CACHE:
        _BUILD_CACHE[key] = build_program(cfg, sched)
    nc = _BUILD_CACHE[key]

    res = run_bass_kernel_spmd(nc, in_maps, core_ids=list(range(cfg.NCORES)),
                               trace=TRACE)
    LAST_RESULTS = res
    return assemble(cfg, res.results)


# revision 26
# speedup vs baseline: 1.1453x; 1.1453x over previous
"""Distributed GCN(4-layer) + LSTM readout kernel for 8 TRN2 NeuronCores.

Self-contained: hardcodes the problem shapes (N=50000, E=800000, D=H=128,
G=500 graphs x L=100 nodes, C=10) and the 8-way sharding.

Strategy (v3)
-------------
- Nodes sharded contiguously across 8 cores at graph boundaries
  (6300 x4 + 6200 x4); per-graph LSTM readout is purely local.
- Per GCN layer each core computes u = t @ W for its shard, writes the slab
  to DRAM and AllGathers the 8 slabs into a replicated bf16 table.
- Edge aggregation as PSUM matmul accumulation over 128-edge chunks:
  psum[f,d] += sum_e GX[e,f] * S[e,d].  GX rows come from a dma_gather of
  the table by src.  S is a host-precomputed "staircase" (one-hot rows
  scaled by a[dst]^2 for layers 0-2 / a[dst] for layer 3), which folds the
  dst-side GCN normalization and the next layer's a-prescale into the
  matmul: the psum is relu'd straight into the next layer's input.
  Self-loops enter as a diag(a^2 | a) matmul of the local slab block.
- dma_gather indices are int16, so the table is addressed in two halves
  (cores 0-3 / 4-7) and per-block edge lists are split accordingly.
- LSTM: x-projections for all timesteps are big matmuls done right after
  layer 4 (gate bias folded in during the psum->SBUF copy).  The recurrence
  uses bf16 W_hh, gates reordered [i,f,o,g], and sigmoid-only activations
  (tanh(x) = 2*sigmoid(2x)-1) to avoid activation-table reloads.
"""
import dataclasses
import numpy as np
import ml_dtypes

import concourse.bass as bass
import concourse.mybir as mybir
import concourse.tile as tile
from concourse import bacc
from concourse.bass_utils import run_bass_kernel_spmd

F32 = mybir.dt.float32
BF16 = mybir.dt.bfloat16
I16 = mybir.dt.int16
P = 128

TRACE = False          # set True (e.g. from test.py) to profile
LAST_RESULTS = None    # BassKernelResults of the last run (for profiling)
DEBUG_DUMPS = False    # add t1/z4 debug outputs


@dataclasses.dataclass
class Config:
    N: int = 50000
    E: int = 800000
    D: int = 128
    H: int = 128
    L: int = 100
    C: int = 10
    NCORES: int = 8
    GROUP_BLOCKS: int = 3  # dst blocks per gather super-group
    PIPE: int = 2          # groups of gather lookahead

    def __post_init__(self):
        assert self.D == 128 and self.H == 128
        base = (self.N // self.NCORES) // self.L * self.L
        hi = base + self.L
        n_hi = (self.N - base * self.NCORES) // self.L
        self.sizes = [hi] * n_hi + [base] * (self.NCORES - n_hi)
        assert sum(self.sizes) == self.N
        self.offs = np.concatenate([[0], np.cumsum(self.sizes)]).astype(np.int64)
        self.S_PAD = hi
        self.NBLK = -(-self.S_PAD // P)
        self.SLAB = self.NBLK * P
        self.THALF = (self.NCORES // 2) * self.SLAB
        assert self.THALF <= 32768, "int16 gather index overflow"
        self.NG = self.S_PAD // self.L
        self.G = self.N // self.L


def preprocess(cfg, x, edge_index, Ws, bs, W_ih, W_hh, b_ih, b_hh,
               lin_W, lin_b):
    N = cfg.N
    src = np.asarray(edge_index[0], np.int64)
    dst = np.asarray(edge_index[1], np.int64)
    deg = (np.bincount(dst, minlength=N) + 1.0).astype(np.float32)
    a = (1.0 / np.sqrt(deg)).astype(np.float32)

    shard_of = np.searchsorted(cfg.offs[1:], np.arange(N), side="right")
    trow = shard_of * cfg.SLAB + (np.arange(N) - cfg.offs[shard_of])

    e_core = shard_of[dst]
    e_half = (trow[src] >= cfg.THALF).astype(np.int64)
    e_tix = (trow[src] - e_half * cfg.THALF).astype(np.int64)
    e_blk = ((dst - cfg.offs[e_core]) // P).astype(np.int64)
    e_seg = ((dst - cfg.offs[e_core]) % P).astype(np.int64)

    order = np.lexsort((dst, e_blk, e_half, e_core))
    src_s = src[order]
    dst_s = dst[order]
    core_s, half_s, tix_s, blk_s, seg_s = (
        arr[order] for arr in (e_core, e_half, e_tix, e_blk, e_seg))

    counts = np.zeros((cfg.NCORES, 2, cfg.NBLK), np.int64)
    np.add.at(counts, (core_s, half_s, blk_s), 1)
    chunks = -(-counts.max(axis=0) // P)
    cA, cB = chunks[0], chunks[1]

    groups = []
    for g0 in range(0, cfg.NBLK, cfg.GROUP_BLOCKS):
        groups.append(list(range(g0, min(g0 + cfg.GROUP_BLOCKS, cfg.NBLK))))

    TA = int(cA.sum()) * P
    TB = int(cB.sum()) * P
    run_off = np.zeros((cfg.NCORES, 2, cfg.NBLK), np.int64)
    run_off.reshape(-1)[1:] = np.cumsum(counts.reshape(-1))[:-1]

    # LSTM gate reorder [i, f, g, o] -> [i, f, o, g] so sigmoids are
    # contiguous; fold b_ih + b_hh into the x-projection copy.
    perm = [0, 1, 3, 2]
    H = cfg.H
    WihR = np.concatenate([np.asarray(W_ih, np.float32)[q * H:(q + 1) * H]
                           for q in perm], axis=0)
    WhhR = np.concatenate([np.asarray(W_hh, np.float32)[q * H:(q + 1) * H]
                           for q in perm], axis=0)
    bgR = np.stack([(np.asarray(b_ih, np.float32)
                     + np.asarray(b_hh, np.float32))[q * H:(q + 1) * H]
                    for q in perm], axis=0)  # [4, H]

    in_maps = []
    for c in range(cfg.NCORES):
        idx_flat = {0: np.zeros(TA, np.int64), 1: np.zeros(TB, np.int64)}
        seg_flat = {0: np.full(TA, -1, np.int64),
                    1: np.full(TB, -1, np.int64)}
        adst_flat = {0: np.zeros(TA, np.float32), 1: np.zeros(TB, np.float32)}
        for h_i, tot_c in enumerate((cA, cB)):
            pos = 0
            for b in range(cfg.NBLK):
                n = int(counts[c, h_i, b])
                o = int(run_off[c, h_i, b])
                idx_flat[h_i][pos:pos + n] = tix_s[o:o + n]
                seg_flat[h_i][pos:pos + n] = seg_s[o:o + n]
                adst_flat[h_i][pos:pos + n] = a[dst_s[o:o + n]]
                pos += int(tot_c[b]) * P

        idxA = np.zeros((P, max(TA // 16, 1)), np.int16)
        idxB = np.zeros((P, max(TB // 16, 1)), np.int16)
        if TA:
            idxA[:] = np.tile(idx_flat[0].reshape(-1, 16).T.astype(np.int16),
                              (8, 1))
        if TB:
            idxB[:] = np.tile(idx_flat[1].reshape(-1, 16).T.astype(np.int16),
                              (8, 1))

        # staircases: [nch, 128(edge), 128(dst)] one-hot rows scaled by
        # a[dst]^2 (layers 0-2) or a[dst] (layer 3); pad rows all-zero.
        stair = {}
        for h_i, T in ((0, TA), (1, TB)):
            nch = max(T // P, 1)
            s2 = np.zeros((nch, P, P), np.float32)
            s1 = np.zeros((nch, P, P), np.float32)
            if T:
                posv = np.arange(T)
                valid = seg_flat[h_i] >= 0
                cc = posv[valid] // P
                pp = posv[valid] % P
                dd = seg_flat[h_i][valid]
                w1 = adst_flat[h_i][valid]
                s1[cc, pp, dd] = w1
                s2[cc, pp, dd] = w1 * w1
            stair[(h_i, 2)] = np.ascontiguousarray(
                s2.transpose(1, 0, 2).reshape(P, -1)).astype(ml_dtypes.bfloat16)
            stair[(h_i, 1)] = np.ascontiguousarray(
                s1.transpose(1, 0, 2).reshape(P, -1)).astype(ml_dtypes.bfloat16)

        o, s = int(cfg.offs[c]), int(cfg.sizes[c])
        a_loc = np.zeros(cfg.SLAB, np.float32)
        a_loc[:s] = a[o:o + s]

        # diag blocks: rhs [node-in-block(K), dst] = diag(a^2 | a)
        dia2 = np.zeros((cfg.NBLK, P, P), np.float32)
        dia1 = np.zeros((cfg.NBLK, P, P), np.float32)
        ar = np.arange(P)
        for b in range(cfg.NBLK):
            av = a_loc[b * P:(b + 1) * P]
            dia2[b, ar, ar] = av * av
            dia1[b, ar, ar] = av
        dia2 = np.ascontiguousarray(
            dia2.transpose(1, 0, 2).reshape(P, -1)).astype(ml_dtypes.bfloat16)
        dia1 = np.ascontiguousarray(
            dia1.transpose(1, 0, 2).reshape(P, -1)).astype(ml_dtypes.bfloat16)

        xT = np.zeros((P, cfg.SLAB), np.float32)
        xT[:, :s] = np.asarray(x[o:o + s], np.float32).T
        a_b = np.ones((P, cfg.SLAB), ml_dtypes.bfloat16)
        a_b[:, :s] = np.tile(a[o:o + s], (P, 1)).astype(ml_dtypes.bfloat16)

        m = {
            "xT": xT, "a_b": a_b,
            "idxA": idxA, "idxB": idxB,
            "stA2": stair[(0, 2)], "stB2": stair[(1, 2)],
            "stA1": stair[(0, 1)], "stB1": stair[(1, 1)],
            "dia2": dia2, "dia1": dia1,
            "WihT": np.ascontiguousarray(WihR.T).astype(ml_dtypes.bfloat16),
            "WhhT": np.ascontiguousarray(WhhR.T).astype(ml_dtypes.bfloat16),
            "bg": np.ascontiguousarray(bgR.T),  # [H, 4] f32
            "linW": np.asarray(lin_W, np.float32).astype(ml_dtypes.bfloat16),
            "linb": np.tile(np.asarray(lin_b, np.float32), (P, 1)),
        }
        for li in range(4):
            m[f"W{li}"] = np.asarray(Ws[li], np.float32).astype(
                ml_dtypes.bfloat16)
        in_maps.append(m)

    sched = dict(cA=cA.astype(np.int64), cB=cB.astype(np.int64),
                 groups=groups, TA=TA, TB=TB,
                 bias_zero=all(not np.any(np.asarray(b)) for b in bs))
    return in_maps, sched


def build_program(cfg, sched, trn_type="TRN2", debug=False):
    nc = bacc.Bacc(trn_type, target_bir_lowering=False, debug=debug,
                   num_devices=cfg.NCORES, num_swdge_queues=4)
    cA, cB, groups = sched["cA"], sched["cB"], sched["groups"]
    TA, TB = sched["TA"], sched["TB"]
    assert sched["bias_zero"], "nonzero GCN bias not supported in v3"
    SLAB, NBLK, THALF, NG = cfg.SLAB, cfg.NBLK, cfg.THALF, cfg.NG
    NGRP = len(groups)

    def din(name, shape, dt):
        return nc.dram_tensor(name, shape, dt, kind="ExternalInput")

    xT_d = din("xT", [P, SLAB], F32)
    a_d = din("a_b", [P, SLAB], BF16)
    idxA_d = din("idxA", [P, max(TA // 16, 1)], I16)
    idxB_d = din("idxB", [P, max(TB // 16, 1)], I16)
    stA_d = {2: din("stA2", [P, max(TA, 1)], BF16),
             1: din("stA1", [P, max(TA, 1)], BF16)}
    stB_d = {2: din("stB2", [P, max(TB, 1)], BF16),
             1: din("stB1", [P, max(TB, 1)], BF16)}
    dia_d = {2: din("dia2", [P, NBLK * P], BF16),
             1: din("dia1", [P, NBLK * P], BF16)}
    W_d = [din(f"W{li}", [P, P], BF16) for li in range(4)]
    WihT_d = din("WihT", [P, 4 * P], BF16)
    WhhT_d = din("WhhT", [P, 4 * P], BF16)
    bg_d = din("bg", [P, 4], F32)
    linW_d = din("linW", [P, cfg.C], BF16)
    linb_d = din("linb", [P, cfg.C], F32)
    out_d = nc.dram_tensor("out", [NG, cfg.C], F32, kind="ExternalOutput")
    if DEBUG_DUMPS:
        t1_d = nc.dram_tensor("t1_dbg", [P, SLAB], BF16,
                              kind="ExternalOutput")
        z4_d = nc.dram_tensor("z4_dbg", [P, SLAB], BF16,
                              kind="ExternalOutput")
        gx_d = nc.dram_tensor("gx_dbg", [P, 4 * SLAB], BF16,
                              kind="ExternalOutput")

    rg = [list(range(cfg.NCORES))]
    qsem = [nc.alloc_semaphore(f"gq{q}") for q in range(4)]

    with tile.TileContext(nc) as tc:
        with tc.tile_pool(name="dram", bufs=1, space="DRAM") as dpool, \
             tc.tile_pool(name="const", bufs=1) as cpool, \
             tc.tile_pool(name="state", bufs=1) as spool, \
             tc.tile_pool(name="work", bufs=2) as wpool, \
             tc.tile_pool(name="gwork", bufs=3) as gpool, \
             tc.tile_pool(name="psum", bufs=4, space="PSUM") as ppool:

            def cload(dram, shape, dt, tag):
                t = cpool.tile(shape, dt, tag=tag)
                nc.sync.dma_start(t[:], dram[:])
                return t

            W_t = [cload(W_d[i], [P, P], BF16, f"c_W{i}") for i in range(4)]
            WihT_t = cload(WihT_d, [P, 4 * P], BF16, "c_wih")
            WhhT_t = cload(WhhT_d, [P, 4 * P], BF16, "c_whh")
            bg_t = cload(bg_d, [P, 4], F32, "c_bg")
            linW_t = cload(linW_d, [P, cfg.C], BF16, "c_linw")
            linb_t = cload(linb_d, [P, cfg.C], F32, "c_linb")

            a_t = spool.tile([P, SLAB], BF16, tag="a")
            nc.sync.dma_start(a_t[:], a_d[:])

            t_even = spool.tile([P, SLAB], BF16, tag="t_even")
            t_big = spool.tile([P, SLAB], BF16, tag="t_big")
            gx_nodes = spool.tile([P, 4 * SLAB], BF16, tag="gx_nodes")
            slab_t = spool.tile([P, SLAB], BF16, tag="slab")

            for j in range(0, SLAB, 512):
                w = min(512, SLAB - j)
                xc = wpool.tile([P, 512], F32, tag="xchunk")
                nc.sync.dma_start(xc[:, :w], xT_d[:, j:j + w])
                nc.vector.tensor_tensor(
                    out=t_even[:, j:j + w], in0=xc[:, :w], in1=a_t[:, j:j + w],
                    op=mybir.AluOpType.mult)

            for li in range(4):
                cur = t_even if li % 2 == 0 else t_big
                nxt = t_big if li % 2 == 0 else t_even
                sca = 2 if li < 3 else 1
                stA_li, stB_li, dia_li = stA_d[sca], stB_d[sca], dia_d[sca]

                slab_dram = dpool.tile([SLAB, P], BF16, tag="slab_dram",
                                       bufs=2)
                table_dram = dpool.tile([cfg.NCORES * SLAB, P], BF16,
                                        addr_space="Shared", tag="table",
                                        bufs=2)

                # ---- u = t @ W -> slab [node, feat] ----
                for k in range(NBLK):
                    pu = ppool.tile([P, P], F32, tag="mm", space="PSUM")
                    nc.tensor.matmul(pu[:], lhsT=cur[:, k * P:(k + 1) * P],
                                     rhs=W_t[li][:], start=True, stop=True)
                    nc.vector.tensor_copy(out=slab_t[:, k * P:(k + 1) * P],
                                          in_=pu[:])
                nc.sync.dma_start(
                    slab_dram[:].rearrange("(b p) f -> p b f", p=P),
                    slab_t[:].rearrange("p (b f) -> p b f", f=P))
                nc.gpsimd.collective_compute(
                    "AllGather", mybir.AluOpType.bypass,
                    replica_groups=rg,
                    ins=[slab_dram[:]],
                    outs=[table_dram[:]],
                )

                # ---- edge aggregation: software-pipelined groups ----
                st_off = [0, 0]
                pend = []

                def stage_prep(gi):
                    blks = groups[gi]
                    q = gi % 4
                    cnts = {0: int(cA[blks].sum()), 1: int(cB[blks].sum())}
                    gx, st = {}, {}
                    for half, idxd, std in ((0, idxA_d, stA_li),
                                            (1, idxB_d, stB_li)):
                        ncnt = cnts[half]
                        if ncnt == 0:
                            continue
                        off = st_off[half]
                        it = gpool.tile([P, ncnt * 8], I16, tag=f"idx{half}")
                        nc.sync.dma_start(
                            it[:], idxd[:, off * 8:(off + ncnt) * 8])
                        s_t = gpool.tile([P, ncnt, P], BF16, tag=f"st{half}")
                        nc.sync.dma_start(
                            s_t[:],
                            std[:, off * P:(off + ncnt) * P]
                            .rearrange("p (c f) -> p c f", f=P))
                        g = gpool.tile([P, ncnt, P], BF16, tag=f"gx{half}")
                        nc.gpsimd.dma_gather(
                            out_ap=g[:],
                            in_ap=table_dram[half * THALF:(half + 1) * THALF, :],
                            idxs_ap=it[:],
                            num_idxs=ncnt * P,
                            num_idxs_reg=ncnt * P,
                            elem_size=P,
                            single_packet=False,
                            queue_num=q,
                        )
                        st_off[half] += ncnt
                        gx[half] = g
                        st[half] = s_t
                    dg = gpool.tile([P, len(blks), P], BF16, tag="diag")
                    b0 = blks[0]
                    nc.sync.dma_start(
                        dg[:],
                        dia_li[:, b0 * P:(b0 + len(blks)) * P]
                        .rearrange("p (c f) -> p c f", f=P))
                    pend.append((blks, gx, st, dg, cnts))

                def stage_fire(gi):
                    blks, gx, st, dg, cnts = pend.pop(0)
                    ca_in, cb_in = 0, 0
                    for bj, b in enumerate(blks):
                        pb = ppool.tile([P, P], F32, tag="mm", space="PSUM")
                        na, nb_ = int(cA[b]), int(cB[b])
                        nc.tensor.matmul(pb[:],
                                         lhsT=slab_t[:, b * P:(b + 1) * P],
                                         rhs=dg[:, bj, :], start=True,
                                         stop=(na + nb_ == 0))
                        done = 0
                        for half, cnt, base in ((0, na, ca_in),
                                                (1, nb_, cb_in)):
                            for ci in range(cnt):
                                col = base + ci
                                done += 1
                                nc.tensor.matmul(
                                    pb[:], lhsT=gx[half][:, col, :],
                                    rhs=st[half][:, col, :],
                                    start=False, stop=(done == na + nb_))
                        ca_in += na
                        cb_in += nb_
                        nc.scalar.activation(
                            out=nxt[:, b * P:(b + 1) * P], in_=pb[:],
                            func=mybir.ActivationFunctionType.Relu)

                for gi in range(NGRP + cfg.PIPE):
                    if gi < NGRP:
                        stage_prep(gi)
                    if gi >= cfg.PIPE:
                        stage_fire(gi - cfg.PIPE)

                if DEBUG_DUMPS and li == 0:
                    nc.sync.dma_start(t1_d[:], nxt[:])

            # ---- LSTM x-projections (z4 = t_even), bias folded in ----
            z4 = t_even
            if DEBUG_DUMPS:
                nc.sync.dma_start(z4_d[:], z4[:])
            for q in range(4):
                for j in range(0, SLAB, 512):
                    w = min(512, SLAB - j)
                    pgx = ppool.tile([P, 512], F32, tag="proj", space="PSUM",
                                     bufs=2)
                    nc.tensor.matmul(
                        pgx[:, :w], lhsT=WihT_t[:, q * P:(q + 1) * P],
                        rhs=z4[:, j:j + w], start=True, stop=True)
                    nc.scalar.activation(
                        out=gx_nodes[:, q * SLAB + j:q * SLAB + j + w],
                        in_=pgx[:, :w],
                        func=mybir.ActivationFunctionType.Identity,
                        bias=bg_t[:, q:q + 1])

            if DEBUG_DUMPS:
                nc.sync.dma_start(gx_d[:], gx_nodes[:])

            # ---- LSTM recurrence: sigmoid-only activations ----
            c_t = spool.tile([P, NG], F32, tag="c")
            h_t = spool.tile([P, NG], BF16, tag="h")
            nc.vector.memset(c_t[:], 0.0)
            nc.vector.memset(h_t[:], 0.0)

            gx3 = gx_nodes[:].rearrange("p (q n) -> p q n", q=4)
            SIG = mybir.ActivationFunctionType.Sigmoid
            for t in range(cfg.L):
                pg = ppool.tile([P, 4 * NG], F32, tag="lstm", space="PSUM",
                                bufs=2)
                for q in range(4):
                    nc.tensor.matmul(
                        pg[:, q * NG:(q + 1) * NG],
                        lhsT=WhhT_t[:, q * P:(q + 1) * P],
                        rhs=h_t[:], start=True, stop=True)
                gsum = wpool.tile([P, 4 * NG], F32, tag="gsum")
                nc.vector.tensor_tensor(
                    out=gsum[:].rearrange("p (q n) -> p q n", q=4),
                    in0=pg[:].rearrange("p (q n) -> p q n", q=4),
                    in1=gx3[:, :, t:cfg.S_PAD:cfg.L],
                    op=mybir.AluOpType.add)
                # gates [i | f | o | g]
                s_ifo = wpool.tile([P, 3 * NG], F32, tag="sifo")
                nc.scalar.activation(out=s_ifo[:], in_=gsum[:, :3 * NG],
                                     func=SIG)
                s_g = wpool.tile([P, NG], F32, tag="sg")
                nc.scalar.activation(out=s_g[:], in_=gsum[:, 3 * NG:],
                                     func=SIG, scale=2.0)
                i_t = s_ifo[:, :NG]
                f_t = s_ifo[:, NG:2 * NG]
                o_t = s_ifo[:, 2 * NG:]
                t1 = wpool.tile([P, NG], F32, tag="t1")
                nc.vector.tensor_tensor(out=t1[:], in0=i_t, in1=s_g[:],
                                        op=mybir.AluOpType.mult)
                ig = wpool.tile([P, NG], F32, tag="ig")
                nc.vector.scalar_tensor_tensor(
                    out=ig[:], in0=t1[:], scalar=2.0, in1=i_t,
                    op0=mybir.AluOpType.mult, op1=mybir.AluOpType.subtract)
                fc = wpool.tile([P, NG], F32, tag="fc")
                nc.vector.tensor_tensor(out=fc[:], in0=f_t, in1=c_t[:],
                                        op=mybir.AluOpType.mult)
                nc.vector.tensor_tensor(out=c_t[:], in0=fc[:], in1=ig[:],
                                        op=mybir.AluOpType.add)
                s_c = wpool.tile([P, NG], F32, tag="sc")
                nc.scalar.activation(out=s_c[:], in_=c_t[:], func=SIG,
                                     scale=2.0)
                t2 = wpool.tile([P, NG], F32, tag="t2")
                nc.vector.tensor_tensor(out=t2[:], in0=o_t, in1=s_c[:],
                                        op=mybir.AluOpType.mult)
                nc.vector.scalar_tensor_tensor(
                    out=h_t[:], in0=t2[:], scalar=2.0, in1=o_t,
                    op0=mybir.AluOpType.mult, op1=mybir.AluOpType.subtract)

            po = ppool.tile([P, cfg.C], F32, tag="lstm", space="PSUM",
                            bufs=2)
            nc.tensor.matmul(po[:NG, :], lhsT=h_t[:, :NG], rhs=linW_t[:],
                             start=True, stop=True)
            os_ = wpool.tile([P, cfg.C], F32, tag="outs")
            nc.vector.tensor_tensor(out=os_[:NG, :], in0=po[:NG, :],
                                    in1=linb_t[:NG, :],
                                    op=mybir.AluOpType.add)
            nc.sync.dma_start(out_d[:], os_[:NG, :])

    nc.compile()
    return nc


def assemble(cfg, results):
    out = np.zeros((cfg.G, cfg.C), np.float32)
    for c in range(cfg.NCORES):
        g0 = int(cfg.offs[c]) // cfg.L
        ng = cfg.sizes[c] // cfg.L
        out[g0:g0 + ng] = results[c]["out"][:ng]
    return out


_BUILD_CACHE = {}


def kernel(x, edge_index, batch, W1, b1, W2, b2, W3, b3, W4, b4,
           W_ih, W_hh, b_ih, b_hh, lin_W, lin_b):
    global LAST_RESULTS
    cfg = Config()
    x = np.asarray(x, np.float32)
    edge_index = np.asarray(edge_index, np.int64)
    Ws = [np.asarray(w, np.float32) for w in (W1, W2, W3, W4)]
    bs = [np.asarray(b, np.float32) for b in (b1, b2, b3, b4)]

    in_maps, sched = preprocess(
        cfg, x, edge_index, Ws, bs,
        np.asarray(W_ih, np.float32), np.asarray(W_hh, np.float32),
        np.asarray(b_ih, np.float32), np.asarray(b_hh, np.float32),
        np.asarray(lin_W, np.float32), np.asarray(lin_b, np.float32))

    key = (sched["TA"], sched["TB"], tuple(sched["cA"]), tuple(sched["cB"]),
           sched["bias_zero"], DEBUG_DUMPS)
    if key not in _BUILD_CACHE:
        _BUILD_CACHE[key] = build_program(cfg, sched)
    nc = _BUILD_CACHE[key]

    res = run_bass_kernel_spmd(nc, in_maps, core_ids=list(range(cfg.NCORES)),
                               trace=TRACE)
    LAST_RESULTS = res
    return assemble(cfg, res.results)
